# revision 1
# baseline (speedup 1.0000x reference)
"""Trainium2 Bass kernel for HandDecoder-style GNN message passing.

Math (per batch element b):
  f = relu(MLP3([feature, coords]))                        # [N, C1=32]
  t1[i,j,h] = relu(a[j,h] + kb1[h] - a[i,h]),  a = coords @ kw1    # [N,N,8]
  t2[i,j,k] = relu(sum_h t1[i,j,h] kw2[h,k] + kb2[k])             # [N,N,16]
  g[j,k,d]  = sum_c f[j,c] kw3[k, c*16+d]                          # [N,16,16]
  out[i,d]  = relu(sum_{j,k} t2[i,j,k] g[j,k,d] + sum_c F[c] kb3[c*16+d])
  (F[c] = sum_j f[j,c])
This is algebraically identical to the reference (which materializes the
[N,N,C1*C2] pairwise kernel tensor) but ~15x less compute and no giant
intermediate. Data-parallel over batch: 4 batch elements per core, 8 cores.
"""

import sys
import numpy as np

for _p in ("/opt/trn_rl_repo",):
    if _p not in sys.path:
        sys.path.insert(0, _p)

import concourse.bass as bass
import concourse.bacc as bacc
import concourse.mybir as mybir
import concourse.tile as tile
from concourse.bass_utils import run_bass_kernel_spmd

B, N = 32, 128
C0, C1, C2 = 64, 32, 16
NCORES = 8
BPC = B // NCORES          # batches per core = 4
F32 = mybir.dt.float32
RELU = mybir.ActivationFunctionType.Relu
COPY = mybir.ActivationFunctionType.Copy

_CACHED_NC = None


def build_nc(stage=5):
    import os
    stage = int(os.environ.get("KSTAGE", stage))
    nc = bacc.Bacc("TRN2", target_bir_lowering=False, debug=False,
                   num_devices=NCORES)

    xT = nc.dram_tensor("xT", [67, BPC * N], F32, kind="ExternalInput").ap()
    c4T = nc.dram_tensor("c4T", [4, BPC * N], F32, kind="ExternalInput").ap()
    sel = nc.dram_tensor("sel", [8, BPC * 8 * 128], F32, kind="ExternalInput").ap()
    wb1 = nc.dram_tensor("wb1", [67, 100], F32, kind="ExternalInput").ap()
    wb2 = nc.dram_tensor("wb2", [128, 401], F32, kind="ExternalInput").ap()
    one512 = nc.dram_tensor("one512", [1, BPC * N], F32, kind="ExternalInput").ap()
    out_d = nc.dram_tensor("out", [N, BPC, C2], F32, kind="ExternalOutput").ap()
    g_dram = nc.dram_tensor("gscr", [BPC * 32768], F32).ap()
    a_dram = nc.dram_tensor("ascr", [4096], F32).ap()

    with tile.TileContext(nc) as tc:
        with (
            tc.tile_pool(name="const", bufs=1) as cpool,
            tc.tile_pool(name="work", bufs=1) as wpool,
            tc.tile_pool(name="ps_misc", bufs=2, space=bass.MemorySpace.PSUM) as pmisc,
            tc.tile_pool(name="ps_t1", bufs=2, space=bass.MemorySpace.PSUM) as pt1,
            tc.tile_pool(name="ps_t2", bufs=2, space=bass.MemorySpace.PSUM) as pt2,
        ):
            # ---- load inputs ----
            xT_s = cpool.tile([67, BPC * N], F32, tag="xT")
            c4T_s = cpool.tile([4, BPC * N], F32, tag="c4T")
            wb1_s = cpool.tile([67, 100], F32, tag="wb1")
            wb2_s = cpool.tile([128, 401], F32, tag="wb2")
            lhsT_all = cpool.tile([9, BPC * 8 * 128], F32, tag="lhsT")
            rhs9 = cpool.tile([9, BPC * N], F32, tag="rhs9")
            nc.sync.dma_start(xT_s[:], xT)
            nc.sync.dma_start(c4T_s[:], c4T)
            nc.sync.dma_start(lhsT_all[0:8, :], sel)
            nc.sync.dma_start(wb1_s[:], wb1)
            nc.sync.dma_start(wb2_s[:], wb2)
            nc.sync.dma_start(rhs9[8:9, :], one512)
            ones128 = cpool.tile([1, 128], F32, tag="ones")
            nc.vector.memset(ones128[:], 1.0)

            # weight-blob slices
            dw1 = wb1_s[0:67, 0:32]
            dw2 = wb1_s[0:32, 32:48]
            dw3 = wb1_s[0:16, 48:80]
            kw1n4 = wb1_s[0:4, 80:88]     # [-kw1; 0]
            kw1b4 = wb1_s[0:4, 88:96]     # [kw1; kb1]
            db1 = wb1_s[0:32, 96:97]
            db2 = wb1_s[0:16, 97:98]
            db3 = wb1_s[0:32, 98:99]
            kb2t = wb2_s[0:128, 0:1]
            kw2bd = wb2_s[0:128, 1:129]   # block-diag kw2, duplicated halves
            kw3p = wb2_s[0:32, 129:385]
            kb3r = wb2_s[0:32, 385:401]

            # ---- a-stage: a_neg = -(coords @ kw1), transposed [8,(b,i)] ----
            aneg_ps = pmisc.tile([8, BPC * N], F32, tag="m")
            nc.tensor.matmul(aneg_ps[:], kw1n4, c4T_s[:])
            nc.scalar.activation(rhs9[0:8, :], aneg_ps[:], COPY)

            # ---- a2b = coords @ kw1 + kb1, row-major [j, (b,h)] ----
            a2b_ps = pmisc.tile([128, BPC * 8], F32, tag="m")
            for b in range(BPC):
                nc.tensor.matmul(a2b_ps[:, b * 8:(b + 1) * 8],
                                 c4T_s[0:4, b * N:(b + 1) * N], kw1b4)
            a2b_sb = wpool.tile([128, BPC * 8], F32, tag="a2b")
            nc.scalar.activation(a2b_sb[:], a2b_ps[:], COPY)
            # scatter a2b into row 8 of lhsT_all via a DRAM bounce
            # (keeps every SBUF-side DMA AP partition-leading):
            # ascr[(jc*16+jl)*32 + b*8 + h] = a2b_sb[jc*16+jl, b*8+h]  (row-major)
            # lhsT_all[8, b*1024 + jc*128 + jl*8 + h] <- ascr 3-dim gather
            nc.sync.dma_start(a_dram, a2b_sb[:])
            src = a_dram.rearrange("(j b h) -> b j h", j=128, h=8)
            nc.sync.dma_start(lhsT_all[8:9, :], src)

            # ---- decode MLP: fT [32, (b,n)] ----
            h1 = wpool.tile([32, BPC * N], F32, tag="h1")
            h2 = wpool.tile([16, BPC * N], F32, tag="h2")
            fT = wpool.tile([32, BPC * N], F32, tag="fT")
            d1_ps = pmisc.tile([32, BPC * N], F32, tag="m")
            nc.tensor.matmul(d1_ps[:], dw1, xT_s[:])
            nc.scalar.activation(h1[:], d1_ps[:], RELU, bias=db1)
            d2_ps = pmisc.tile([16, BPC * N], F32, tag="m")
            nc.tensor.matmul(d2_ps[:], dw2, h1[:])
            nc.scalar.activation(h2[:], d2_ps[:], RELU, bias=db2)
            d3_ps = pmisc.tile([32, BPC * N], F32, tag="m")
            nc.tensor.matmul(d3_ps[:], dw3, h2[:])
            nc.scalar.activation(fT[:], d3_ps[:], RELU, bias=db3)

            if stage == 1:
                dbg = wpool.tile([128, BPC * C2], F32, tag="dbg")
                nc.vector.memset(dbg[:], 0.0)
                nc.vector.tensor_scalar(dbg[0:32, 0:64], fT[:, 0:64], 0.0, None,
                                        mybir.AluOpType.add)
                nc.sync.dma_start(out_d, dbg[:])
            if stage >= 2:
                # ---- t1 stage: 64 matmuls [9,64]x[9,128] -> relu ----
                # tile idx2=(b*8+jc)*2+half: partitions p=jl*8+h (jl within half),
                # free i. All tiles are 64-partition, base 0 (HW dislikes base-64
                # PE operands).
                t1_sb = []
                t1_ps_tiles = []
                for grp in range(16):         # 4 chunk-halves per group
                    ps = pt1.tile([64, 512], F32, tag="t1ps")
                    t1_ps_tiles.append(ps)
                    sb = wpool.tile([64, 512], F32, tag=f"t1sb{grp}")
                    t1_sb.append(sb)
                for b in range(BPC):
                    for jc in range(8):
                        for half in range(2):
                            idx2 = (b * 8 + jc) * 2 + half
                            grp, q = divmod(idx2, 4)
                            col = (b * 8 + jc) * 128 + half * 64
                            nc.tensor.matmul(
                                t1_ps_tiles[grp][:, q * 128:(q + 1) * 128],
                                lhsT_all[0:9, col:col + 64],
                                rhs9[0:9, b * N:(b + 1) * N])
                for grp in range(16):
                    if grp % 2 == 0:
                        nc.scalar.activation(t1_sb[grp][:], t1_ps_tiles[grp][:], RELU)
                    else:
                        nc.vector.tensor_scalar(
                            t1_sb[grp][:], t1_ps_tiles[grp][:], 0.0, None,
                            mybir.AluOpType.max)

                if stage == 2:
                    dbg = wpool.tile([128, BPC * C2], F32, tag="dbg")
                    nc.scalar.activation(dbg[0:64, :], t1_sb[15][:, 0:BPC * C2], COPY)
                    nc.scalar.activation(dbg[64:128, :], t1_sb[14][:, 0:BPC * C2], COPY)
                    nc.sync.dma_start(out_d, dbg[:])
            if stage >= 3:
                # ---- t2 stage: 64 matmuls lhsT=kw2bd [64,128] ----
                # chunk c2 = jc*2 + half covers j in [c2*8, c2*8+8);
                # output partition p = jl*16 + k.
                t2_sb = []
                t2_ps_tiles = []
                for grp in range(16):
                    ps = pt2.tile([128, 512], F32, tag="t2ps")
                    t2_ps_tiles.append(ps)
                    sb = wpool.tile([128, 512], F32, tag=f"t2sb{grp}")
                    t2_sb.append(sb)
                kw2bd0 = wb2_s[0:64, 1:129]
                for b in range(BPC):
                    for jc in range(8):
                        for half in range(2):
                            idx = (b * 8 + jc) * 2 + half     # 0..63
                            grp, q = divmod(idx, 4)
                            nc.tensor.matmul(
                                t2_ps_tiles[grp][:, q * 128:(q + 1) * 128],
                                kw2bd0,
                                t1_sb[grp][:, q * 128:(q + 1) * 128])
                for grp in range(16):
                    if grp % 2 == 0:
                        nc.scalar.activation(t2_sb[grp][:], t2_ps_tiles[grp][:],
                                             RELU, bias=kb2t)
                    else:
                        nc.vector.tensor_scalar(
                            t2_sb[grp][:], t2_ps_tiles[grp][:], kb2t, 0.0,
                            mybir.AluOpType.add, mybir.AluOpType.max)

            if stage == 3:
                dbg = wpool.tile([128, BPC * C2], F32, tag="dbg")
                nc.scalar.activation(dbg[:], t2_sb[15][:, 0:BPC * C2], COPY)
                nc.sync.dma_start(out_d, dbg[:])
            if stage >= 4:
                # ---- g stage (+ bias2) ----
                with tc.tile_pool(name="ps_g", bufs=1,
                                  space=bass.MemorySpace.PSUM) as pg:
                    g_ps = pg.tile([128, BPC * 256], F32, tag="g")
                    g_rm = wpool.tile([128, BPC * 256], F32, tag="grm")
                    g_all = wpool.tile([128, BPC * 256], F32, tag="gall")
                    for b in range(BPC):
                        nc.tensor.matmul(g_ps[:, b * 256:(b + 1) * 256],
                                         fT[0:32, b * N:(b + 1) * N], kw3p)
                    for b in range(BPC):
                        if b % 2 == 0:
                            nc.scalar.activation(g_rm[:, b * 256:(b + 1) * 256],
                                                 g_ps[:, b * 256:(b + 1) * 256], COPY)
                        else:
                            nc.vector.tensor_copy(g_rm[:, b * 256:(b + 1) * 256],
                                                  g_ps[:, b * 256:(b + 1) * 256])
                    # reshape: g_all[(jl k), b*256 + c*16 + d] = g_rm[c*8+jl, b*256+k*16+d]
                    # bounce through DRAM: SBUF partition-regroup DMAs are limited
                    # (partition dim must lead / span 128), DRAM APs are linear.
                    # A) g_dram[b*32768 + j*256 + k*16 + d] = g_rm row-major
                    # B) g_all[(jl*16+k), b*256+c2*16+d] <- gather (jlk, c2, d) 3-dim AP
                    for b in range(BPC):
                        eng = nc.sync if b % 2 == 0 else nc.scalar
                        eng.dma_start(g_dram[b * 32768:(b + 1) * 32768],
                                      g_rm[:, b * 256:(b + 1) * 256])
                    for b in range(BPC):
                        eng = nc.sync if b % 2 == 0 else nc.scalar
                        dstB = g_all[:, b * 256:(b + 1) * 256].rearrange(
                            "p (c d) -> p c d", d=16)
                        srcB = g_dram[b * 32768:(b + 1) * 32768].rearrange(
                            "(c jk d) -> jk c d", jk=128, d=16)
                        eng.dma_start(dstB, srcB)

                    # bias2[b,d] = sum_c F[b,c] kb3r[c,d];  F = rowsum of f
                    F_sb = wpool.tile([32, BPC], F32, tag="F")
                    for b in range(BPC):
                        nc.vector.tensor_reduce(F_sb[:, b:b + 1],
                                                fT[0:32, b * N:(b + 1) * N],
                                                mybir.AxisListType.X,
                                                mybir.AluOpType.add)
                    bias2_ps = pmisc.tile([1, BPC * C2], F32, tag="m")
                    for b in range(BPC):
                        nc.tensor.matmul(bias2_ps[0:1, b * C2:(b + 1) * C2],
                                         F_sb[0:32, b:b + 1], kb3r)
                    bias2_sb = wpool.tile([1, BPC * C2], F32, tag="b2")
                    nc.scalar.activation(bias2_sb[:], bias2_ps[:], COPY)

            if stage == 4:
                dbg = wpool.tile([128, BPC * C2], F32, tag="dbg")
                nc.scalar.activation(dbg[:], g_all[:, 0:BPC * C2], COPY)
                nc.sync.dma_start(out_d, dbg[:])
            if stage >= 5:
                # ---- final contraction ----
                with tc.tile_pool(name="ps_out", bufs=1,
                                  space=bass.MemorySpace.PSUM) as pout:
                    out_ps = pout.tile([128, BPC * C2], F32, tag="o")
                    for b in range(BPC):
                        for c2 in range(16):
                            idx = (b * 8 + c2 // 2) * 2 + c2 % 2
                            grp, q = divmod(idx, 4)
                            nc.tensor.matmul(
                                out_ps[:, b * C2:(b + 1) * C2],
                                t2_sb[grp][:, q * 128:(q + 1) * 128],
                                g_all[:, b * 256 + c2 * 16: b * 256 + (c2 + 1) * 16],
                                start=(c2 == 0), stop=False)
                        nc.tensor.matmul(out_ps[:, b * C2:(b + 1) * C2],
                                         ones128[0:1, 0:128],
                                         bias2_sb[0:1, b * C2:(b + 1) * C2],
                                         start=False, stop=True)
                    out_sb = wpool.tile([128, BPC * C2], F32, tag="osb")
                    nc.scalar.activation(out_sb[:], out_ps[:], RELU)
                    nc.sync.dma_start(out_d, out_sb[:])

    nc.compile()
    return nc


def _host_inputs(feature, coordinates_v, dw1, db1, dw2, db2, dw3, db3,
                 kw1, kb1, kw2, kb2, kw3, kb3):
    """Per-core input maps. Pure layout transforms, no FLOPs."""
    f32 = np.float32
    # wb1: small weights packed column-wise into a [67, 100] blob
    wb1 = np.zeros((67, 100), f32)
    wb1[0:67, 0:32] = dw1
    wb1[0:32, 32:48] = dw2
    wb1[0:16, 48:80] = dw3
    wb1[0:3, 80:88] = -kw1
    wb1[0:3, 88:96] = kw1
    wb1[3, 88:96] = kb1
    wb1[0:32, 96] = db1
    wb1[0:16, 97] = db2
    wb1[0:32, 98] = db3
    # wb2: kb2 tiled, block-diag kw2 (dup halves), permuted kw3, kb3
    wb2 = np.zeros((128, 401), f32)
    wb2[:, 0] = np.tile(kb2, 8)
    bd = np.zeros((64, 128), f32)
    for jl in range(8):
        bd[jl * 8:(jl + 1) * 8, jl * 16:(jl + 1) * 16] = kw2
    wb2[0:64, 1:129] = bd
    wb2[64:128, 1:129] = bd
    wb2[0:32, 129:385] = kw3.reshape(16, 32, 16).transpose(1, 0, 2).reshape(32, 256)
    wb2[0:32, 385:401] = kb3.reshape(32, 16)
    # selector rows for the t1 matmul
    cols = np.arange(BPC * 8 * 128)
    sel = (cols[None, :] % 8 == np.arange(8)[:, None]).astype(f32)

    in_maps = []
    for c in range(NCORES):
        fe = feature[c * BPC:(c + 1) * BPC]          # [4, 64]
        co = coordinates_v[c * BPC:(c + 1) * BPC]    # [4, 128, 3]
        xT = np.empty((67, BPC * N), f32)
        c4T = np.empty((4, BPC * N), f32)
        for b in range(BPC):
            xT[0:64, b * N:(b + 1) * N] = fe[b][:, None]
            xT[64:67, b * N:(b + 1) * N] = co[b].T
            c4T[0:3, b * N:(b + 1) * N] = co[b].T
        c4T[3, :] = 1.0
        in_maps.append({"xT": np.ascontiguousarray(xT),
                        "c4T": np.ascontiguousarray(c4T),
                        "sel": sel, "wb1": wb1, "wb2": wb2,
                        "one512": np.ones((1, BPC * N), f32)})
    return in_maps


def kernel(**inputs):
    global _CACHED_NC
    if _CACHED_NC is None:
        _CACHED_NC = build_nc()
    nc = _CACHED_NC
    in_maps = _host_inputs(
        np.asarray(inputs["feature"]), np.asarray(inputs["coordinates_v"]),
        np.asarray(inputs["dw1"]), np.asarray(inputs["db1"]),
        np.asarray(inputs["dw2"]), np.asarray(inputs["db2"]),
        np.asarray(inputs["dw3"]), np.asarray(inputs["db3"]),
        np.asarray(inputs["kw1"]), np.asarray(inputs["kb1"]),
        np.asarray(inputs["kw2"]), np.asarray(inputs["kb2"]),
        np.asarray(inputs["kw3"]), np.asarray(inputs["kb3"]))
    res = run_bass_kernel_spmd(nc, in_maps, list(range(NCORES)))
    out = np.empty((B, N, C2), np.float32)
    for c in range(NCORES):
        # per-core out is [N(i), BPC(b), C2(d)]
        out[c * BPC:(c + 1) * BPC] = res.results[c]["out"].transpose(1, 0, 2)
    return out



# revision 15
# speedup vs baseline: 2.6172x; 2.6172x over previous
"""Trainium2 Bass kernel for HandDecoder-style GNN message passing.

Math (per batch element b, N=128 nodes):
  f = relu(MLP3([feature, coords]))                          # [N, C1=32]
  t1[i,j,h] = relu(a[j,h] + kb1[h] - a[i,h]),  a = coords @ kw1   # [N,N,8]
  t2[i,j,k] = relu(sum_h t1[i,j,h] kw2[h,k] + kb2[k])             # [N,N,16]
  g[j,k,d]  = sum_c f[j,c] kw3[k, c*16+d]                          # [N,16,16]
  out[i,d]  = relu(sum_{j,k} t2[i,j,k] g[j,k,d] + sum_c F[c] kb3[c*16+d])
  (F[c] = sum_j f[j,c])

v2 design (vs the fp32 baseline at ~133us):
  - all matmul operands bf16 (PE: 1 cyc/row vs 4 for fp32; tol is 2e-2)
  - t1 computed on Act/DVE/Pool engines via activation(in=-a_rep, bias=a2b)
    instead of 64 PE matmuls (frees the bottleneck engine, kills the
    selector input + PSUM->SBUF copies)
  - t2: 64 bf16 matmuls, K=128 (block-diag kw2 with zero rows), M=128, N=128
  - final contraction swapped: outT[d,i] = sum g_chunk^T t2_chunk so the
    per-batch bias2 lands on the partition axis and rides the drain act
  - g permute (j,(k,d)) -> ((jl,k),(b,c2,d)) via one scatter-write +
    one clean-read DRAM bounce, scheduled early so it hides under t1/t2
Data-parallel over batch: 4 batch elements per core, 8 cores.
"""

import sys
import numpy as np

for _p in ("/opt/trn_rl_repo",):
    if _p not in sys.path:
        sys.path.insert(0, _p)

import concourse.bass as bass
import concourse.bacc as bacc
import concourse.mybir as mybir
import concourse.tile as tile
from concourse.bass_utils import run_bass_kernel_spmd

import ml_dtypes

B, N = 32, 128
C0, C1, C2 = 64, 32, 16
NCORES = 8
BPC = B // NCORES          # batches per core = 4
F32 = mybir.dt.float32
BF16 = mybir.dt.bfloat16
RELU = mybir.ActivationFunctionType.Relu
COPY = mybir.ActivationFunctionType.Copy
BNP = ml_dtypes.bfloat16

_CACHED_NC = None


def build_nc(stage=5):
    import os
    stage = int(os.environ.get("KSTAGE", stage))
    nc = bacc.Bacc("TRN2", target_bir_lowering=False, debug=False,
                   num_devices=NCORES)

    # -------- DRAM I/O --------
    # wb_bf: all bf16 weights packed column-wise into one [128, 744] blob:
    #   [0:256]  W2: W2[jl*8+h, half*128+jl8*16+k] = kw2[h,k] if jl==half*8+jl8
    #   [256:512] kw3p rows 0:32: kw3p[c, k*16+d] = kw3[k, c*16+d]
    #   [512:544] dw1 rows 0:67
    #   [544:560] dw2 rows 0:32
    #   [560:592] dw3 rows 0:16
    #   [592:600] kw1n4 rows 0:4: [-kw1; 0]
    #   [600:728] kw1blk rows 0:64: kw1blk[jl*4+x, jl2*8+h] = kw1b[x,h]*(jl==jl2)
    #   [728:744] kb3r rows 0:32: kb3r[c, d] = kb3[c*16+d]
    #   [744:872] selrep rows 0:8: selrep[r, jl*8+h] = (h==r)
    wb_bf = nc.dram_tensor("wb_bf", [128, 872], BF16, kind="ExternalInput").ap()
    # wb_f32: fp32 per-partition bias columns [128, 4]:
    #   col0 db1 (rows 0:32), col1 db2 (rows 0:16), col2 db3 (rows 0:32),
    #   col3 kb2t (rows 0:128, kb2 tiled: kb2t[jl*16+k] = kb2[k])
    wb_f32 = nc.dram_tensor("wb_f32", [128, 4], F32, kind="ExternalInput").ap()
    # xT: [feature bcast; coords^T] per (b,i) column, bf16
    xT = nc.dram_tensor("xT", [67, BPC * N], BF16, kind="ExternalInput").ap()
    # c4T: rows 0:3 coords^T, row 3 unused-by-aneg (lhsT row 3 is 0)
    c4T = nc.dram_tensor("c4T", [4, BPC * N], BF16, kind="ExternalInput").ap()
    # c4T2[jl*4+x, b*8+jc] = coords[b, jc*16+jl, x] (x<3), 1.0 (x==3)
    c4T2 = nc.dram_tensor("c4T2", [64, BPC * 8], BF16, kind="ExternalInput").ap()
    # out: [d, (b,i)] fp32; host transposes back
    out_d = nc.dram_tensor("out", [C2, BPC * N], F32, kind="ExternalOutput").ap()
    # g bounce scratch, target-major: [(jl*16+k), b, c2, d]
    g_dram = nc.dram_tensor("gscr", [128 * BPC * 256], BF16).ap()

    with tile.TileContext(nc) as tc:
        with (
            tc.tile_pool(name="const", bufs=1) as cpool,
            tc.tile_pool(name="work", bufs=1) as wpool,
            tc.tile_pool(name="ps_misc", bufs=2, space=bass.MemorySpace.PSUM) as pmisc,
            tc.tile_pool(name="ps_rep", bufs=1, space=bass.MemorySpace.PSUM) as prep,
            tc.tile_pool(name="ps_t2", bufs=2, space=bass.MemorySpace.PSUM) as pt2,
        ):
            # ---- load inputs ----
            wb = cpool.tile([128, 872], BF16, tag="wb")
            wf = cpool.tile([128, 4], F32, tag="wf")
            xT_s = cpool.tile([67, BPC * N], BF16, tag="xT")
            c4T_s = cpool.tile([4, BPC * N], BF16, tag="c4T")
            c4T2_s = cpool.tile([64, BPC * 8], BF16, tag="c4T2")
            nc.sync.dma_start(wb[:], wb_bf)
            nc.sync.dma_start(wf[:], wb_f32)
            nc.sync.dma_start(xT_s[:], xT)
            nc.sync.dma_start(c4T_s[:], c4T)
            nc.sync.dma_start(c4T2_s[:], c4T2)

            W2 = wb[0:128, 0:256]
            kw3p = wb[0:32, 256:512]
            dw1 = wb[0:67, 512:544]
            dw2 = wb[0:32, 544:560]
            dw3 = wb[0:16, 560:592]
            kw1n4 = wb[0:4, 592:600]
            kw1blk = wb[0:64, 600:728]
            kb3r = wb[0:32, 728:744]
            selrep = wb[0:8, 744:872]
            db1 = wf[0:32, 0:1]
            db2 = wf[0:16, 1:2]
            db3 = wf[0:32, 2:3]
            kb2t = wf[0:128, 3:4]

            # ---- a-stage: aneg = -(coords @ kw1), [8, (b,i)] ----
            aneg_ps = pmisc.tile([8, BPC * N], F32, tag="m")
            nc.tensor.matmul(aneg_ps[:], kw1n4, c4T_s[:])
            aneg_sb = wpool.tile([8, BPC * N], BF16, tag="aneg")
            nc.scalar.activation(aneg_sb[:], aneg_ps[:], COPY)
            # rep[(jl,h), (b,i)] = aneg[h, (b,i)]; copied to SBUF so the t1
            # ops can run on Pool too (GPSIMD cannot access PSUM).
            rep_ps = prep.tile([128, BPC * N], F32, tag="rep")
            nc.tensor.matmul(rep_ps[:], selrep, aneg_sb[:])
            rep_sb = wpool.tile([128, BPC * N], F32, tag="repsb")
            nc.vector.tensor_copy(rep_sb[:], rep_ps[:])

            # ---- a2bT[(jl,h), (b,jc)] = coords@kw1 + kb1 at j=jc*16+jl ----
            a2bT_ps = pmisc.tile([128, BPC * 8], F32, tag="m")
            nc.tensor.matmul(a2bT_ps[:], kw1blk, c4T2_s[:])
            a2bT_sb = wpool.tile([128, BPC * 8], F32, tag="a2bT")
            nc.vector.tensor_copy(a2bT_sb[:], a2bT_ps[:])

            # ---- decode MLP: fT [32, (b,i)] bf16 ----
            h1 = wpool.tile([32, BPC * N], BF16, tag="h1")
            h2 = wpool.tile([16, BPC * N], BF16, tag="h2")
            fT = wpool.tile([32, BPC * N], BF16, tag="fT")
            d1_ps = pmisc.tile([32, BPC * N], F32, tag="m")
            nc.tensor.matmul(d1_ps[:], dw1, xT_s[:])
            nc.scalar.activation(h1[:], d1_ps[:], RELU, bias=db1)
            d2_ps = pmisc.tile([16, BPC * N], F32, tag="m")
            nc.tensor.matmul(d2_ps[:], dw2, h1[:])
            nc.scalar.activation(h2[:], d2_ps[:], RELU, bias=db2)
            d3_ps = pmisc.tile([32, BPC * N], F32, tag="m")
            nc.tensor.matmul(d3_ps[:], dw3, h2[:])
            nc.scalar.activation(fT[:], d3_ps[:], RELU, bias=db3)

            if stage == 1:
                dbg = wpool.tile([C2, BPC * N], F32, tag="dbg")
                nc.vector.memset(dbg[:], 0.0)
                nc.vector.tensor_scalar(dbg[0:16, 0:512], fT[0:16, :], 0.0,
                                        None, mybir.AluOpType.add)
                nc.sync.dma_start(out_d, dbg[:])

            engs = (nc.scalar, nc.vector, nc.gpsimd)

            def relu_bias(eng, out, in_, bias):
                # out = relu(in_ + bias), bias is a per-partition [P,1] AP
                if eng is nc.scalar:
                    eng.activation(out, in_, RELU, bias=bias)
                else:
                    eng.tensor_scalar(out, in_, bias, 0.0,
                                      mybir.AluOpType.add, mybir.AluOpType.max)

            if stage >= 4:
                # ---- g stage + bounce (early: hides under t1/t2) ----
                # g_ps[b][j, (k,d)] = sum_c fT[c, (b,j)] kw3p[c, (k,d)]
                with tc.tile_pool(name="ps_g", bufs=1,
                                  space=bass.MemorySpace.PSUM) as pg:
                    g_ps = pg.tile([128, BPC * 256], F32, tag="g")
                    # g_sb[j, (k, b, d)]: drains shuffle (k,d) -> (k, b, d)
                    g_sb = wpool.tile([128, BPC * 256], BF16, tag="gsb")
                    g_sb_v = g_sb[:].rearrange("p (k b d) -> p b k d",
                                               k=16, b=BPC, d=16)
                    for b in range(BPC):
                        nc.tensor.matmul(g_ps[:, b * 256:(b + 1) * 256],
                                         fT[0:32, b * N:(b + 1) * N], kw3p)
                    for b in range(BPC):
                        eng = (nc.scalar, nc.vector)[b % 2]
                        srcv = g_ps[:, b * 256:(b + 1) * 256].rearrange(
                            "p (k d) -> p k d", d=16)
                        dstv = g_sb_v[:, b]
                        if eng is nc.scalar:
                            eng.activation(dstv, srcv, COPY)
                        else:
                            eng.tensor_copy(dstv, srcv)
                # clean write: g_dram[j*1024 + k*64 + b*16 + d] (j-major).
                # Key: j-stride (1024) = 16 * k-stride (64), so on the read
                # side partition p=(jl*16+k) maps affinely (stride 64).
                nc.sync.dma_start(g_dram.rearrange("(j f) -> j f", j=128),
                                  g_sb[:])
                # gather read: G_big[(jl*16+k), (c2, b, d)]
                #   <- g_dram[c2*8192 + p*64 + (b*16+d)]
                g_all = wpool.tile([128, BPC * 256], BF16, tag="gall")
                nc.sync.dma_start(
                    g_all[:].rearrange("p (c f) -> p c f", c=16, f=64),
                    g_dram.rearrange("(c p f) -> p c f", c=16, p=128, f=64))

                # bias2T[d, b] = sum_c kb3r[c, d] F[c, b],  F = rowsum of f
                F_f32 = wpool.tile([32, BPC], F32, tag="Ff")
                for b in range(BPC):
                    nc.vector.tensor_reduce(F_f32[:, b:b + 1],
                                            fT[0:32, b * N:(b + 1) * N],
                                            mybir.AxisListType.X,
                                            mybir.AluOpType.add)
                F_sb = wpool.tile([32, BPC], BF16, tag="F")
                nc.vector.tensor_copy(F_sb[:], F_f32[:])
                bias2_ps = pmisc.tile([16, BPC], F32, tag="m")
                nc.tensor.matmul(bias2_ps[:], kb3r, F_sb[:])
                bias2_sb = wpool.tile([16, BPC], F32, tag="b2")
                nc.vector.tensor_copy(bias2_sb[:], bias2_ps[:])

            if stage >= 2:
                # ---- t1: 32 activations on Act/DVE/Pool ----
                # t1_sb[b][jc][(jl,h), i] = relu(a2bT[(jl,h),(b,jc)] - a[h,(b,i)])
                t1_sb = {}
                for jc in range(8):
                    for b in range(BPC):
                        t = wpool.tile([128, N], BF16, name=f"t1_{b}_{jc}",
                                       tag=f"t1_{b}_{jc}")
                        t1_sb[(b, jc)] = t
                        relu_bias(engs[(jc * BPC + b) % 3], t[:],
                                  rep_sb[:, b * N:(b + 1) * N],
                                  a2bT_sb[:, b * 8 + jc:b * 8 + jc + 1])

            if stage == 2:
                dbg = wpool.tile([C2, BPC * N], F32, tag="dbg")
                nc.vector.tensor_scalar(dbg[:], t1_sb[(0, 0)][0:16, 0:512], 0.0,
                                        None, mybir.AluOpType.add)
                nc.sync.dma_start(out_d, dbg[:])

            if stage >= 3:
                # ---- t2: 64 matmuls + 32 drains ----
                # T2[b][c2=2jc+half][(jl8,k), i], drained into t2b[b] columns
                # c2*128+i as bf16.
                t2b = [wpool.tile([128, 2048], BF16, name=f"t2b{b}",
                                  tag=f"t2b{b}") for b in range(BPC)]
                for jc2 in range(4):
                    for b in range(BPC):
                        ps = pt2.tile([128, 512], F32, tag="t2ps")
                        for q in range(2):
                            for half in range(2):
                                nc.tensor.matmul(
                                    ps[:, (q * 2 + half) * 128:
                                       (q * 2 + half + 1) * 128],
                                    W2[:, half * 128:(half + 1) * 128],
                                    t1_sb[(b, jc2 * 2 + q)][:])
                        relu_bias((nc.scalar, nc.vector)[(jc2 * BPC + b) % 2],
                                  t2b[b][:, jc2 * 512:(jc2 + 1) * 512],
                                  ps[:], kb2t)

            if stage == 3:
                dbg = wpool.tile([C2, BPC * N], F32, tag="dbg")
                nc.vector.tensor_scalar(dbg[:], t2b[0][0:16, 0:512], 0.0,
                                        None, mybir.AluOpType.add)
                nc.sync.dma_start(out_d, dbg[:])
            if stage == 4:
                dbg = wpool.tile([C2, BPC * N], F32, tag="dbg")
                nc.vector.tensor_scalar(dbg[:], g_all[0:16, 0:512], 0.0,
                                        None, mybir.AluOpType.add)
                nc.sync.dma_start(out_d, dbg[:])

            if stage >= 5:
                # ---- final: outT[d, (b,i)] = sum_c2 g_chunk^T @ t2_chunk ----
                with tc.tile_pool(name="ps_out", bufs=1,
                                  space=bass.MemorySpace.PSUM) as pout:
                    ot_ps = pout.tile([16, BPC * N], F32, tag="ot")
                    for b in range(BPC):
                        for c2 in range(16):
                            nc.tensor.matmul(
                                ot_ps[:, b * N:(b + 1) * N],
                                g_all[:, c2 * 64 + b * 16:
                                      c2 * 64 + (b + 1) * 16],
                                t2b[b][:, c2 * 128:(c2 + 1) * 128],
                                start=(c2 == 0), stop=(c2 == 15))
                    out_sb = wpool.tile([16, BPC * N], F32, tag="osb")
                    for b in range(BPC):
                        nc.scalar.activation(out_sb[:, b * N:(b + 1) * N],
                                             ot_ps[:, b * N:(b + 1) * N],
                                             RELU, bias=bias2_sb[:, b:b + 1])
                    nc.sync.dma_start(out_d, out_sb[:])

    nc.compile()
    return nc


def _host_inputs(feature, coordinates_v, dw1, db1, dw2, db2, dw3, db3,
                 kw1, kb1, kw2, kb2, kw3, kb3):
    """Per-core input maps. Pure layout transforms, no FLOPs."""
    f32 = np.float32
    wb = np.zeros((128, 872), f32)
    # W2
    for half in range(2):
        for jl8 in range(8):
            jl = half * 8 + jl8
            wb[jl * 8:(jl + 1) * 8, half * 128 + jl8 * 16:
               half * 128 + (jl8 + 1) * 16] = kw2
    # kw3p[c, k*16+d] = kw3[k, c*16+d]
    wb[0:32, 256:512] = np.asarray(kw3).reshape(16, 32, 16).transpose(
        1, 0, 2).reshape(32, 256)
    wb[0:67, 512:544] = dw1
    wb[0:32, 544:560] = dw2
    wb[0:16, 560:592] = dw3
    wb[0:3, 592:600] = -np.asarray(kw1)
    # kw1blk
    kw1b = np.concatenate([np.asarray(kw1), np.asarray(kb1)[None, :]], 0)  # [4,8]
    for jl in range(16):
        wb[jl * 4:(jl + 1) * 4, 600 + jl * 8:600 + (jl + 1) * 8] = kw1b
    wb[0:32, 728:744] = np.asarray(kb3).reshape(32, 16)
    # selrep
    cols = np.arange(128)
    wb[0:8, 744:872] = (cols[None, :] % 8 == np.arange(8)[:, None]).astype(f32)
    wb = wb.astype(BNP)

    wf = np.zeros((128, 4), f32)
    wf[0:32, 0] = db1
    wf[0:16, 1] = db2
    wf[0:32, 2] = db3
    wf[:, 3] = np.tile(kb2, 8)

    in_maps = []
    for c in range(NCORES):
        fe = feature[c * BPC:(c + 1) * BPC]          # [4, 64]
        co = coordinates_v[c * BPC:(c + 1) * BPC]    # [4, 128, 3]
        xT = np.empty((67, BPC * N), f32)
        c4T = np.zeros((4, BPC * N), f32)
        for b in range(BPC):
            xT[0:64, b * N:(b + 1) * N] = fe[b][:, None]
            xT[64:67, b * N:(b + 1) * N] = co[b].T
            c4T[0:3, b * N:(b + 1) * N] = co[b].T
        # c4T2[jl*4+x, b*8+jc] = co[b, jc*16+jl, x] (x<3), 1 (x==3)
        c4T2 = np.empty((64, BPC * 8), f32)
        v = co.reshape(BPC, 8, 16, 3)                # [b, jc, jl, x]
        for jl in range(16):
            c4T2[jl * 4:jl * 4 + 3, :] = v[:, :, jl, :].transpose(
                2, 0, 1).reshape(3, BPC * 8)
            c4T2[jl * 4 + 3, :] = 1.0
        in_maps.append({"xT": xT.astype(BNP), "c4T": c4T.astype(BNP),
                        "c4T2": c4T2.astype(BNP), "wb_bf": wb, "wb_f32": wf})
    return in_maps


def kernel(**inputs):
    global _CACHED_NC
    if _CACHED_NC is None:
        _CACHED_NC = build_nc()
    nc = _CACHED_NC
    in_maps = _host_inputs(
        np.asarray(inputs["feature"]), np.asarray(inputs["coordinates_v"]),
        np.asarray(inputs["dw1"]), np.asarray(inputs["db1"]),
        np.asarray(inputs["dw2"]), np.asarray(inputs["db2"]),
        np.asarray(inputs["dw3"]), np.asarray(inputs["db3"]),
        np.asarray(inputs["kw1"]), np.asarray(inputs["kb1"]),
        np.asarray(inputs["kw2"]), np.asarray(inputs["kb2"]),
        np.asarray(inputs["kw3"]), np.asarray(inputs["kb3"]))
    res = run_bass_kernel_spmd(nc, in_maps, list(range(NCORES)))
    out = np.empty((B, N, C2), np.float32)
    for c in range(NCORES):
        # per-core out is [C2(d), BPC(b)*N(i)]
        r = res.results[c]["out"].reshape(C2, BPC, N)
        out[c * BPC:(c + 1) * BPC] = r.transpose(1, 2, 0)
    return out
